# revision 21
# baseline (speedup 1.0000x reference)
"""BERT self-attention (B=4, S=2048, E=768, H=12) on 8 TRN2 NeuronCores.

Sharding: (batch, head-half) — core c handles batch c//2, heads 6*(c%2)..+6.
Each core is fully independent (no collectives).

Host-side prep (in kernel()): per-core shard slicing plus layout/precision
prep — hidden/W transposed to put the contraction dim on partitions, Wq/bq
pre-scaled by 1/sqrt(D), attention_mask folded into domain mask, matmul
operands fed as bf16 (what the device would cast them to anyway).

Device-side structure (per core):
  - projections (bf16): qT,kT in [o,m] layout; v in [m,o] layout augmented
    with a ones column per head (softmax denominators via the PV matmul).
  - scores^T[k,q] = kT.T @ qT, two heads row-packed per PE pass (d=64 each)
    into one f32 PSUM tile [128, 1024].
  - one ACT pass per k-chunk: exp(scores) PSUM -> SBUF bf16. This is the
    kernel bottleneck (~1.1 us per 128x1024 tile, ScalarE ~92% busy).
  - masks enter via E_T = exp(maskT) (ACT, interleaved with the first
    k-loop), multiplied in at bf16 2x on DVE: prod = exp_s * E_T.
  - PV: ctx_u^T[65,q] = v_aug.T @ prod accumulated over 16 k-chunks in
    PSUM; row 64 is the softmax denominator.
  - PE-transpose ctx_u^T -> [q,65], divide rows 0..63 by row 64 (DVE
    reciprocal + one broadcast multiply per head), outputs cast-DMA'd
    bf16 -> f32, one DMA per 512-row stripe.

Pipelining: stage-D pools are opened before the projection pools so SBUF/
PSUM regions do not overlap (avoids pool-release stalls); the k-loop emits
the next score pair ahead of the previous chunk's exp/mult/PV; x/W ride
the sync DMA queue while W/mask chunks alternate with the gpsimd queue.

Measured on 8 axon TRN2 cores: ~307 us HW exec, rel L2 err ~7e-3 vs the
f32 reference (bf16 compute).
"""

import sys

if "/opt/trn_rl_repo" not in sys.path:
    sys.path.insert(0, "/opt/trn_rl_repo")

from contextlib import ExitStack

import ml_dtypes
import numpy as np

import concourse.bass as bass
import concourse.tile as tile
from concourse import bacc, mybir
from concourse.bass_utils import run_bass_kernel_spmd
from concourse.masks import make_identity

B, S, E, H = 4, 2048, 768, 12
D = 64
N_CORES = 8
HPC = 6            # heads per core
EC = HPC * D       # 384 embedding cols per core
NIC = E // 128     # 6 contraction chunks
NOC = EC // 128    # 3 output chunks (= head pairs)
NKC = S // 128     # 16 k chunks
QW = 512           # q tile width
NQQ = S // QW      # 4 q chunks

F32 = mybir.dt.float32
BF16 = mybir.dt.bfloat16
Exp = mybir.ActivationFunctionType.Exp


def _bcast_last(ap: bass.AP, n: int) -> bass.AP:
    """Append a step-0 broadcast dim of size n to an AP."""
    return bass.AP(tensor=ap.tensor, offset=ap.offset, ap=[*ap.ap, [0, n]])


def _emit(ctx: ExitStack, tc: tile.TileContext, h):
    nc = tc.nc

    persist = ctx.enter_context(tc.tile_pool(name="persist", bufs=1))
    consts = ctx.enter_context(tc.tile_pool(name="consts", bufs=1))

    # ---- constants ----
    idt = consts.tile([128, 128], BF16)
    make_identity(nc, idt[:])
    bq_sb = consts.tile([128, NOC], F32)
    nc.gpsimd.dma_start(out=bq_sb[:], in_=h["bq"].ap())
    bk_sb = consts.tile([128, NOC], F32)
    nc.gpsimd.dma_start(out=bk_sb[:], in_=h["bk"].ap())
    bv_sb = consts.tile([1, EC], BF16)
    nc.gpsimd.dma_start(out=bv_sb[:], in_=h["bv"].ap())
    ones1 = consts.tile([1, 128], BF16)
    nc.vector.memset(ones1[:], 1.0)

    # ---- persistent activations ----
    qT = persist.tile([128, NOC, S], BF16)        # [o%128, o-chunk, m]
    kT = persist.tile([128, NOC, S], BF16)
    vaug = persist.tile([128, NKC, HPC, D + 4], BF16)  # [m%128, m-chunk, head, d|one]
    ET = persist.tile([128, NKC, S], BF16)        # exp(dmaskT + amask), [k%128, k-chunk, q]

    nc.vector.memset(vaug[:, :, :, D : D + 1], 1.0)

    cstg = ctx.enter_context(tc.tile_pool(name="cstg", bufs=3))

    # stage-D pools open first so their SBUF/PSUM does not overlap the
    # projection pools (avoids release-chain stalls at the phase boundary)
    sps = ctx.enter_context(tc.tile_pool(name="s_psum", bufs=3, space="PSUM"))
    dwork = ctx.enter_context(tc.tile_pool(name="dwork", bufs=4))
    owork = ctx.enter_context(tc.tile_pool(name="owork", bufs=3))

    # ---- stages A+B: load + projections; stage C interleaved ----
    with tc.tile_pool(name="stageAB", bufs=1) as sab, \
         tc.tile_pool(name="proj_psum", bufs=2, space="PSUM") as pps:
        xTb = sab.tile([128, NIC, S], BF16)
        wqb = sab.tile([128, NIC, EC], BF16)
        wkb = sab.tile([128, NIC, EC], BF16)
        wvb = sab.tile([128, NIC, EC], BF16)
        # xT (bf16): straight load on the sync queue
        for c in range(NIC // 2):
            nc.sync.dma_start(
                out=xTb[:, 2 * c : 2 * c + 2, :],
                in_=h["xT"].ap()[c * 256 : (c + 1) * 256, :].rearrange(
                    "(a p) q -> p a q", p=128
                ),
            )
        # W (bf16): gpsimd queue, concurrent with the sync queue
        for name, wtb in (("wqT", wqb), ("wkT", wkb), ("wvT", wvb)):
            for ic in range(NIC):
                nc.gpsimd.dma_start(
                    out=wtb[:, ic, :], in_=h[name].ap()[ic * 128 : (ic + 1) * 128, :]
                )

        def proj_qk(dst, wtb, bias, oc):
            for mq in range(NQQ):
                ps = pps.tile([128, QW], F32, tag="pp")
                for ic in range(NIC):
                    nc.tensor.matmul(
                        ps[:],
                        wtb[:, ic, oc * 128 : (oc + 1) * 128],
                        xTb[:, ic, mq * QW : (mq + 1) * QW],
                        start=(ic == 0),
                        stop=(ic == NIC - 1),
                    )
                nc.vector.tensor_scalar_add(
                    dst[:, oc, mq * QW : (mq + 1) * QW], ps[:], bias[:, oc : oc + 1]
                )

        def proj_v(mc):
            vps_full = pps.tile([128, QW], F32, tag="pp")
            vps = vps_full[:, 0:EC]
            for ic in range(NIC):
                nc.tensor.matmul(
                    vps[:],
                    xTb[:, ic, mc * 128 : (mc + 1) * 128],
                    wvb[:, ic, :],
                    start=(ic == 0),
                    stop=False,
                )
            nc.tensor.matmul(vps[:], ones1[:], bv_sb[:], start=False, stop=True)
            nc.vector.tensor_copy(
                vaug[:, mc, :, 0:D], vps[:].rearrange("p (h d) -> p h d", h=HPC)
            )

        # j=0 needs qT/kT chunk 0 + vaug; emit those first so stage D can start
        proj_qk(qT, wqb, bq_sb, 0)
        proj_qk(kT, wkb, bk_sb, 0)
        for mc in range(NKC):
            proj_v(mc)
        for oc in range(1, NOC):
            proj_qk(qT, wqb, bq_sb, oc)
            proj_qk(kT, wkb, bk_sb, oc)

    # ---- stage C: E_T = exp(maskT), per-k-chunk so the ACT queue drains
    # while the projections still own the PE ----
    for kc in range(NKC):
        dm = cstg.tile([128, S], BF16, tag="dm")
        eng = nc.sync if kc % 2 == 0 else nc.gpsimd
        eng.dma_start(
            out=dm[:], in_=h["dmaskT"].ap()[kc * 128 : (kc + 1) * 128, :]
        )
        nc.scalar.activation(ET[:, kc, :], dm[:], Exp)

    # ---- stage D: attention ----
    with tc.tile_pool(name="tailwork", bufs=7) as twork:

        def tail(S_t, kc, j, qs, ctxA, ctxB):
            ex = dwork.tile([128, 2 * QW], BF16, tag="ex")
            nc.scalar.activation(ex[:], S_t[:], Exp)
            pr = dwork.tile([128, 2 * QW], BF16, tag="pr")
            et_ap = ET[:, kc, qs]
            et_b = bass.AP(
                tensor=et_ap.tensor, offset=et_ap.offset,
                ap=[et_ap.ap[0], [0, 2], *et_ap.ap[1:]],
            )
            nc.vector.tensor_tensor(
                pr[:].rearrange("p (g q) -> p g q", g=2),
                ex[:].rearrange("p (g q) -> p g q", g=2),
                et_b,
                op=mybir.AluOpType.mult,
            )
            nc.tensor.matmul(
                ctxA[:], vaug[:, kc, 2 * j, 0 : D + 1], pr[:, 0:QW],
                start=(kc == 0), stop=(kc == NKC - 1),
            )
            nc.tensor.matmul(
                ctxB[:], vaug[:, kc, 2 * j + 1, 0 : D + 1], pr[:, QW : 2 * QW],
                start=(kc == 0), stop=(kc == NKC - 1),
            )

        for qq in range(NQQ):
            qs = slice(qq * QW, (qq + 1) * QW)
            osb_t = owork.tile([128, 4, EC], BF16, tag="osb")
            csbs = []
            for j in range(NOC):
                with tc.tile_pool(name="ctxp", bufs=1, space="PSUM") as cps:
                    ctxA = cps.tile([D + 1, QW], F32, tag="cA")
                    ctxB = cps.tile([D + 1, QW], F32, tag="cB")
                    prev = None
                    for kc in range(NKC):
                        ks = slice(kc * 128, (kc + 1) * 128)
                        S_t = sps.tile([128, 2 * QW], F32, tag="S")
                        nc.tensor.matmul(
                            S_t[:, 0:QW], kT[0:64, j, ks], qT[0:64, j, qs],
                            start=True, stop=True, tile_position=(0, 0),
                        )
                        nc.tensor.matmul(
                            S_t[:, QW : 2 * QW], kT[64:128, j, ks], qT[64:128, j, qs],
                            start=True, stop=True, tile_position=(64, 0),
                        )
                        if prev is not None:
                            tail(prev[0], prev[1], j, qs, ctxA, ctxB)
                        prev = (S_t, kc)
                    tail(prev[0], prev[1], j, qs, ctxA, ctxB)
                    for cpsum in (ctxA, ctxB):
                        csb = twork.tile([D + 1, QW], BF16, tag="csb")
                        nc.vector.tensor_copy(csb[:], cpsum[:])
                        csbs.append(csb)
            # output tail once per qq stripe (keeps the PE queue clear of
            # transposes at j boundaries)
            with tc.tile_pool(name="tpp", bufs=2, space="PSUM") as tpp:
                for hj, csb in enumerate(csbs):
                    tp = tpp.tile([128, 4, D + 4], BF16, tag="tp")
                    for t in range(4):
                        nc.tensor.transpose(
                            tp[:, t, 0 : D + 1],
                            csb[:, t * 128 : (t + 1) * 128],
                            idt[0 : D + 1, 0 : D + 1],
                        )
                    rc4 = twork.tile([128, 4], F32, tag="rc4")
                    nc.vector.reciprocal(
                        rc4[:], tp[:, :, D : D + 1].rearrange("p a b -> p (a b)")
                    )
                    col = hj * D
                    nc.vector.tensor_tensor(
                        osb_t[:, :, col : col + D],
                        tp[:, :, 0:D],
                        _bcast_last(rc4[:], D),
                        op=mybir.AluOpType.mult,
                    )
            nc.gpsimd.dma_start(
                out=h["out"].ap()[qq * QW : (qq + 1) * QW, :].rearrange(
                    "(t p) e -> p t e", p=128
                ),
                in_=osb_t[:],
            )


def build():
    nc = bacc.Bacc("TRN2", target_bir_lowering=False, debug=False, num_devices=N_CORES)
    h = {
        "xT": nc.dram_tensor("xT", [E, S], BF16, kind="ExternalInput"),
        "wqT": nc.dram_tensor("wqT", [E, EC], BF16, kind="ExternalInput"),
        "wkT": nc.dram_tensor("wkT", [E, EC], BF16, kind="ExternalInput"),
        "wvT": nc.dram_tensor("wvT", [E, EC], BF16, kind="ExternalInput"),
        "bq": nc.dram_tensor("bq", [128, NOC], F32, kind="ExternalInput"),
        "bk": nc.dram_tensor("bk", [128, NOC], F32, kind="ExternalInput"),
        "bv": nc.dram_tensor("bv", [1, EC], BF16, kind="ExternalInput"),
        "dmaskT": nc.dram_tensor("dmaskT", [S, S], BF16, kind="ExternalInput"),
        "out": nc.dram_tensor("out", [S, EC], F32, kind="ExternalOutput"),
    }
    with tile.TileContext(nc) as tc:
        with ExitStack() as ctx:
            _emit(ctx, tc, h)
    nc.compile()
    return nc


def prep_in_maps(inputs):
    hs = np.asarray(inputs["hidden_states"], dtype=np.float32)
    am = np.asarray(inputs["attention_mask"], dtype=np.float32)
    dm = np.asarray(inputs["domain_attn_mask"], dtype=np.float32)
    Wq = np.asarray(inputs["Wq"], dtype=np.float32)
    bq = np.asarray(inputs["bq"], dtype=np.float32)
    Wk = np.asarray(inputs["Wk"], dtype=np.float32)
    bk = np.asarray(inputs["bk"], dtype=np.float32)
    Wv = np.asarray(inputs["Wv"], dtype=np.float32)
    bv = np.asarray(inputs["bv"], dtype=np.float32)

    in_maps = []
    for c in range(N_CORES):
        b = c // 2
        e0 = (c % 2) * EC
        sl = slice(e0, e0 + EC)
        in_maps.append(
            {
                "xT": np.ascontiguousarray(hs[b].T).astype(ml_dtypes.bfloat16),
                "wqT": (np.ascontiguousarray(Wq[sl, :].T) * 0.125).astype(
                    ml_dtypes.bfloat16
                ),
                "wkT": np.ascontiguousarray(Wk[sl, :].T).astype(ml_dtypes.bfloat16),
                "wvT": np.ascontiguousarray(Wv[sl, :].T).astype(ml_dtypes.bfloat16),
                "bq": np.ascontiguousarray((bq[sl] * 0.125).reshape(NOC, 128).T),
                "bk": np.ascontiguousarray(bk[sl].reshape(NOC, 128).T),
                "bv": bv[sl].reshape(1, EC).astype(ml_dtypes.bfloat16),
                "dmaskT": (dm[b, 0].T + am[b, 0, 0, :, None]).astype(ml_dtypes.bfloat16),
            }
        )
    return in_maps


_cached_nc = None


def run(inputs, trace=False):
    global _cached_nc
    if _cached_nc is None:
        _cached_nc = build()
    in_maps = prep_in_maps(inputs)
    res = run_bass_kernel_spmd(
        _cached_nc, in_maps, core_ids=list(range(N_CORES)), trace=trace
    )
    out = np.empty((B, S, E), dtype=np.float32)
    for c in range(N_CORES):
        b = c // 2
        e0 = (c % 2) * EC
        out[b, :, e0 : e0 + EC] = res.results[c]["out"]
    return out, res


def kernel(**inputs) -> np.ndarray:
    return run(inputs)[0]


# revision 22
# speedup vs baseline: 1.0228x; 1.0228x over previous
"""BERT self-attention (B=4, S=2048, E=768, H=12) on 8 TRN2 NeuronCores.

Sharding: (batch, head-half) — core c handles batch c//2, heads 6*(c%2)..+6.
Each core is fully independent (no collectives).

Host-side prep (in kernel()): per-core shard slicing plus layout/precision
prep — hidden/W transposed to put the contraction dim on partitions, Wq/bq
pre-scaled by 1/sqrt(D), attention_mask folded into domain mask, matmul
operands fed as bf16 (what the device would cast them to anyway).

Device-side structure (per core):
  - projections (bf16): qT,kT in [o,m] layout; v in [m,o] layout augmented
    with a ones column per head (softmax denominators via the PV matmul).
  - scores^T[k,q] = kT.T @ qT, two heads row-packed per PE pass (d=64 each)
    into one f32 PSUM tile [128, 1024].
  - one ACT pass per k-chunk: exp(scores) PSUM -> SBUF bf16. This is the
    kernel bottleneck (~1.1 us per 128x1024 tile, ScalarE ~92% busy).
  - masks enter via E_T = exp(maskT) (ACT, interleaved with the first
    k-loop), multiplied in at bf16 2x on DVE: prod = exp_s * E_T.
  - PV: ctx_u^T[65,q] = v_aug.T @ prod accumulated over 16 k-chunks in
    PSUM; row 64 is the softmax denominator.
  - PE-transpose ctx_u^T -> [q,65], divide rows 0..63 by row 64 (DVE
    reciprocal + one broadcast multiply per head), outputs cast-DMA'd
    bf16 -> f32, one DMA per 512-row stripe.

Pipelining: stage-D pools are opened before the projection pools so SBUF/
PSUM regions do not overlap (avoids pool-release stalls); the k-loop emits
the next score pair ahead of the previous chunk's exp/mult/PV; x/W ride
the sync DMA queue while W/mask chunks alternate with the gpsimd queue.

Measured on 8 axon TRN2 cores: ~307 us HW exec, rel L2 err ~7e-3 vs the
f32 reference (bf16 compute).
"""

import sys

if "/opt/trn_rl_repo" not in sys.path:
    sys.path.insert(0, "/opt/trn_rl_repo")

from contextlib import ExitStack

import ml_dtypes
import numpy as np

import concourse.bass as bass
import concourse.tile as tile
from concourse import bacc, mybir
from concourse.bass_utils import run_bass_kernel_spmd
from concourse.masks import make_identity

B, S, E, H = 4, 2048, 768, 12
D = 64
N_CORES = 8
HPC = 6            # heads per core
EC = HPC * D       # 384 embedding cols per core
NIC = E // 128     # 6 contraction chunks
NOC = EC // 128    # 3 output chunks (= head pairs)
NKC = S // 128     # 16 k chunks
QW = 512           # q tile width
NQQ = S // QW      # 4 q chunks

F32 = mybir.dt.float32
BF16 = mybir.dt.bfloat16
Exp = mybir.ActivationFunctionType.Exp


def _bcast_last(ap: bass.AP, n: int) -> bass.AP:
    """Append a step-0 broadcast dim of size n to an AP."""
    return bass.AP(tensor=ap.tensor, offset=ap.offset, ap=[*ap.ap, [0, n]])


def _emit(ctx: ExitStack, tc: tile.TileContext, h):
    nc = tc.nc

    persist = ctx.enter_context(tc.tile_pool(name="persist", bufs=1))
    consts = ctx.enter_context(tc.tile_pool(name="consts", bufs=1))

    # ---- constants ----
    idt = consts.tile([128, 128], BF16)
    make_identity(nc, idt[:])
    bq_sb = consts.tile([128, NOC], F32)
    nc.gpsimd.dma_start(out=bq_sb[:], in_=h["bq"].ap())
    bk_sb = consts.tile([128, NOC], F32)
    nc.gpsimd.dma_start(out=bk_sb[:], in_=h["bk"].ap())
    bv_sb = consts.tile([1, EC], BF16)
    nc.gpsimd.dma_start(out=bv_sb[:], in_=h["bv"].ap())
    ones1 = consts.tile([1, 128], BF16)
    nc.vector.memset(ones1[:], 1.0)

    # ---- persistent activations ----
    qT = persist.tile([128, NOC, S], BF16)        # [o%128, o-chunk, m]
    kT = persist.tile([128, NOC, S], BF16)
    vaug = persist.tile([128, NKC, HPC, D + 4], BF16)  # [m%128, m-chunk, head, d|one]
    ET = persist.tile([128, NKC, S], BF16)        # exp(dmaskT + amask), [k%128, k-chunk, q]

    nc.vector.memset(vaug[:, :, :, D : D + 1], 1.0)

    cstg = ctx.enter_context(tc.tile_pool(name="cstg", bufs=3))

    # stage-D pools open first so their SBUF/PSUM does not overlap the
    # projection pools (avoids release-chain stalls at the phase boundary)
    sps = ctx.enter_context(tc.tile_pool(name="s_psum", bufs=3, space="PSUM"))
    dwork = ctx.enter_context(tc.tile_pool(name="dwork", bufs=4))
    owork = ctx.enter_context(tc.tile_pool(name="owork", bufs=3))

    # ---- stages A+B: load + projections; stage C interleaved ----
    with tc.tile_pool(name="stageAB", bufs=1) as sab, \
         tc.tile_pool(name="proj_psum", bufs=2, space="PSUM") as pps:
        xTb = sab.tile([128, NIC, S], BF16)
        wqb = sab.tile([128, NIC, EC], BF16)
        wkb = sab.tile([128, NIC, EC], BF16)
        wvb = sab.tile([128, NIC, EC], BF16)
        # xT (bf16): straight load on the sync queue
        for c in range(NIC // 2):
            nc.sync.dma_start(
                out=xTb[:, 2 * c : 2 * c + 2, :],
                in_=h["xT"].ap()[c * 256 : (c + 1) * 256, :].rearrange(
                    "(a p) q -> p a q", p=128
                ),
            )
        # W (bf16): gpsimd queue, concurrent with the sync queue
        for name, wtb in (("wqT", wqb), ("wkT", wkb), ("wvT", wvb)):
            for ic in range(NIC):
                nc.gpsimd.dma_start(
                    out=wtb[:, ic, :], in_=h[name].ap()[ic * 128 : (ic + 1) * 128, :]
                )

        def proj_qk(dst, wtb, bias, oc):
            for mq in range(NQQ):
                ps = pps.tile([128, QW], F32, tag="pp")
                for ic in range(NIC):
                    nc.tensor.matmul(
                        ps[:],
                        wtb[:, ic, oc * 128 : (oc + 1) * 128],
                        xTb[:, ic, mq * QW : (mq + 1) * QW],
                        start=(ic == 0),
                        stop=(ic == NIC - 1),
                    )
                nc.vector.tensor_scalar_add(
                    dst[:, oc, mq * QW : (mq + 1) * QW], ps[:], bias[:, oc : oc + 1]
                )

        def proj_v(mc):
            vps_full = pps.tile([128, QW], F32, tag="pp")
            vps = vps_full[:, 0:EC]
            for ic in range(NIC):
                nc.tensor.matmul(
                    vps[:],
                    xTb[:, ic, mc * 128 : (mc + 1) * 128],
                    wvb[:, ic, :],
                    start=(ic == 0),
                    stop=False,
                )
            nc.tensor.matmul(vps[:], ones1[:], bv_sb[:], start=False, stop=True)
            nc.vector.tensor_copy(
                vaug[:, mc, :, 0:D], vps[:].rearrange("p (h d) -> p h d", h=HPC)
            )

        # j=0 needs qT/kT chunk 0 + vaug; emit those first so stage D can start
        proj_qk(qT, wqb, bq_sb, 0)
        proj_qk(kT, wkb, bk_sb, 0)
        for mc in range(NKC):
            proj_v(mc)
        for oc in range(1, NOC):
            proj_qk(qT, wqb, bq_sb, oc)
            proj_qk(kT, wkb, bk_sb, oc)

    # ---- stage C: E_T = exp(maskT), per-k-chunk so the ACT queue drains
    # while the projections still own the PE ----
    for kc in range(NKC):
        dm = cstg.tile([128, S], BF16, tag="dm")
        eng = nc.sync if kc % 2 == 0 else nc.gpsimd
        eng.dma_start(
            out=dm[:], in_=h["dmaskT"].ap()[kc * 128 : (kc + 1) * 128, :]
        )
        nc.scalar.activation(ET[:, kc, :], dm[:], Exp)

    # ---- stage D: attention ----
    with tc.tile_pool(name="tailwork", bufs=7) as twork:

        def tail(S_t, kc, j, qs, ctxA, ctxB):
            ex = dwork.tile([128, 2 * QW], BF16, tag="ex")
            nc.scalar.activation(ex[:], S_t[:], Exp)
            pr = dwork.tile([128, 2 * QW], BF16, tag="pr")
            et_ap = ET[:, kc, qs]
            et_b = bass.AP(
                tensor=et_ap.tensor, offset=et_ap.offset,
                ap=[et_ap.ap[0], [0, 2], *et_ap.ap[1:]],
            )
            nc.vector.tensor_tensor(
                pr[:].rearrange("p (g q) -> p g q", g=2),
                ex[:].rearrange("p (g q) -> p g q", g=2),
                et_b,
                op=mybir.AluOpType.mult,
            )
            nc.tensor.matmul(
                ctxA[:], vaug[:, kc, 2 * j, 0 : D + 1], pr[:, 0:QW],
                start=(kc == 0), stop=(kc == NKC - 1),
            )
            nc.tensor.matmul(
                ctxB[:], vaug[:, kc, 2 * j + 1, 0 : D + 1], pr[:, QW : 2 * QW],
                start=(kc == 0), stop=(kc == NKC - 1),
            )

        for qq in range(NQQ):
            qs = slice(qq * QW, (qq + 1) * QW)
            osb_t = owork.tile([128, 4, EC], BF16, tag="osb")
            for j in range(NOC):
                csbs = []
                with tc.tile_pool(name="ctxp", bufs=1, space="PSUM") as cps:
                    ctxA = cps.tile([D + 1, QW], F32, tag="cA")
                    ctxB = cps.tile([D + 1, QW], F32, tag="cB")
                    prev = None
                    for kc in range(NKC):
                        ks = slice(kc * 128, (kc + 1) * 128)
                        S_t = sps.tile([128, 2 * QW], F32, tag="S")
                        nc.tensor.matmul(
                            S_t[:, 0:QW], kT[0:64, j, ks], qT[0:64, j, qs],
                            start=True, stop=True, tile_position=(0, 0),
                        )
                        nc.tensor.matmul(
                            S_t[:, QW : 2 * QW], kT[64:128, j, ks], qT[64:128, j, qs],
                            start=True, stop=True, tile_position=(64, 0),
                        )
                        if prev is not None:
                            tail(prev[0], prev[1], j, qs, ctxA, ctxB)
                        prev = (S_t, kc)
                    tail(prev[0], prev[1], j, qs, ctxA, ctxB)
                    for cpsum in (ctxA, ctxB):
                        csb = twork.tile([D + 1, QW], BF16, tag="csb")
                        nc.vector.tensor_copy(csb[:], cpsum[:])
                        csbs.append(csb)
                with tc.tile_pool(name="tpp", bufs=2, space="PSUM") as tpp:
                    for hh, csb in enumerate(csbs):
                        tp = tpp.tile([128, 4, D + 4], BF16, tag="tp")
                        for t in range(4):
                            nc.tensor.transpose(
                                tp[:, t, 0 : D + 1],
                                csb[:, t * 128 : (t + 1) * 128],
                                idt[0 : D + 1, 0 : D + 1],
                            )
                        rc4 = twork.tile([128, 4], F32, tag="rc4")
                        nc.vector.reciprocal(
                            rc4[:], tp[:, :, D : D + 1].rearrange("p a b -> p (a b)")
                        )
                        col = (2 * j + hh) * D
                        nc.vector.tensor_tensor(
                            osb_t[:, :, col : col + D],
                            tp[:, :, 0:D],
                            _bcast_last(rc4[:], D),
                            op=mybir.AluOpType.mult,
                        )
            nc.gpsimd.dma_start(
                out=h["out"].ap()[qq * QW : (qq + 1) * QW, :].rearrange(
                    "(t p) e -> p t e", p=128
                ),
                in_=osb_t[:],
            )


def build():
    nc = bacc.Bacc("TRN2", target_bir_lowering=False, debug=False, num_devices=N_CORES)
    h = {
        "xT": nc.dram_tensor("xT", [E, S], BF16, kind="ExternalInput"),
        "wqT": nc.dram_tensor("wqT", [E, EC], BF16, kind="ExternalInput"),
        "wkT": nc.dram_tensor("wkT", [E, EC], BF16, kind="ExternalInput"),
        "wvT": nc.dram_tensor("wvT", [E, EC], BF16, kind="ExternalInput"),
        "bq": nc.dram_tensor("bq", [128, NOC], F32, kind="ExternalInput"),
        "bk": nc.dram_tensor("bk", [128, NOC], F32, kind="ExternalInput"),
        "bv": nc.dram_tensor("bv", [1, EC], BF16, kind="ExternalInput"),
        "dmaskT": nc.dram_tensor("dmaskT", [S, S], BF16, kind="ExternalInput"),
        "out": nc.dram_tensor("out", [S, EC], F32, kind="ExternalOutput"),
    }
    with tile.TileContext(nc) as tc:
        with ExitStack() as ctx:
            _emit(ctx, tc, h)
    nc.compile()
    return nc


def prep_in_maps(inputs):
    hs = np.asarray(inputs["hidden_states"], dtype=np.float32)
    am = np.asarray(inputs["attention_mask"], dtype=np.float32)
    dm = np.asarray(inputs["domain_attn_mask"], dtype=np.float32)
    Wq = np.asarray(inputs["Wq"], dtype=np.float32)
    bq = np.asarray(inputs["bq"], dtype=np.float32)
    Wk = np.asarray(inputs["Wk"], dtype=np.float32)
    bk = np.asarray(inputs["bk"], dtype=np.float32)
    Wv = np.asarray(inputs["Wv"], dtype=np.float32)
    bv = np.asarray(inputs["bv"], dtype=np.float32)

    in_maps = []
    for c in range(N_CORES):
        b = c // 2
        e0 = (c % 2) * EC
        sl = slice(e0, e0 + EC)
        in_maps.append(
            {
                "xT": np.ascontiguousarray(hs[b].T).astype(ml_dtypes.bfloat16),
                "wqT": (np.ascontiguousarray(Wq[sl, :].T) * 0.125).astype(
                    ml_dtypes.bfloat16
                ),
                "wkT": np.ascontiguousarray(Wk[sl, :].T).astype(ml_dtypes.bfloat16),
                "wvT": np.ascontiguousarray(Wv[sl, :].T).astype(ml_dtypes.bfloat16),
                "bq": np.ascontiguousarray((bq[sl] * 0.125).reshape(NOC, 128).T),
                "bk": np.ascontiguousarray(bk[sl].reshape(NOC, 128).T),
                "bv": bv[sl].reshape(1, EC).astype(ml_dtypes.bfloat16),
                "dmaskT": (dm[b, 0].T + am[b, 0, 0, :, None]).astype(ml_dtypes.bfloat16),
            }
        )
    return in_maps


_cached_nc = None


def run(inputs, trace=False):
    global _cached_nc
    if _cached_nc is None:
        _cached_nc = build()
    in_maps = prep_in_maps(inputs)
    res = run_bass_kernel_spmd(
        _cached_nc, in_maps, core_ids=list(range(N_CORES)), trace=trace
    )
    out = np.empty((B, S, E), dtype=np.float32)
    for c in range(N_CORES):
        b = c // 2
        e0 = (c % 2) * EC
        out[b, :, e0 : e0 + EC] = res.results[c]["out"]
    return out, res


def kernel(**inputs) -> np.ndarray:
    return run(inputs)[0]


# revision 30
# speedup vs baseline: 1.0310x; 1.0081x over previous
"""BERT self-attention (B=4, S=2048, E=768, H=12) on 8 TRN2 NeuronCores.

Sharding: (batch, head-half) — core c handles batch c//2, heads 6*(c%2)..+6.
Each core is fully independent (no collectives).

Host-side prep (in kernel()): per-core shard slicing plus layout/precision
prep — hidden/W transposed to put the contraction dim on partitions, Wq/bq
pre-scaled by 1/sqrt(D), attention_mask folded into domain mask, matmul
operands fed as bf16 (what the device would cast them to anyway).

Device-side structure (per core):
  - projections (bf16): qT,kT in [o,m] layout; v in [m,o] layout augmented
    with a ones column per head (softmax denominators via the PV matmul).
  - scores^T[k,q] = kT.T @ qT, two heads row-packed per PE pass (d=64 each)
    into one f32 PSUM tile [128, 1024].
  - one ACT pass per k-chunk: exp(scores) PSUM -> SBUF bf16. This is the
    kernel bottleneck (~1.1 us per 128x1024 tile, ScalarE ~92% busy).
  - masks enter via E_T = exp(maskT) (ACT, interleaved with the first
    k-loop), multiplied in at bf16 2x on DVE: prod = exp_s * E_T.
  - PV: ctx_u^T[65,q] = v_aug.T @ prod accumulated over 16 k-chunks in
    PSUM; row 64 is the softmax denominator.
  - PE-transpose ctx_u^T -> [q,65], divide rows 0..63 by row 64 (DVE
    reciprocal + one broadcast multiply per head), outputs cast-DMA'd
    bf16 -> f32, one DMA per 512-row stripe.

Pipelining: stage-D pools are opened before the projection pools so SBUF/
PSUM regions do not overlap (avoids pool-release stalls); the k-loop emits
the next score pair ahead of the previous chunk's exp/mult/PV; x/W ride
the sync DMA queue while W/mask chunks alternate with the gpsimd queue.

Measured on 8 axon TRN2 cores: ~307 us HW exec, rel L2 err ~7e-3 vs the
f32 reference (bf16 compute).
"""

import sys

if "/opt/trn_rl_repo" not in sys.path:
    sys.path.insert(0, "/opt/trn_rl_repo")

from contextlib import ExitStack

import ml_dtypes
import numpy as np

import concourse.bass as bass
import concourse.tile as tile
from concourse import bacc, mybir
from concourse.bass_utils import run_bass_kernel_spmd
from concourse.masks import make_identity

B, S, E, H = 4, 2048, 768, 12
D = 64
N_CORES = 8
HPC = 6            # heads per core
EC = HPC * D       # 384 embedding cols per core
NIC = E // 128     # 6 contraction chunks
NOC = EC // 128    # 3 output chunks (= head pairs)
NKC = S // 128     # 16 k chunks
QW = 512           # q tile width
NQQ = S // QW      # 4 q chunks

F32 = mybir.dt.float32
BF16 = mybir.dt.bfloat16
Exp = mybir.ActivationFunctionType.Exp


def _bcast_last(ap: bass.AP, n: int) -> bass.AP:
    """Append a step-0 broadcast dim of size n to an AP."""
    return bass.AP(tensor=ap.tensor, offset=ap.offset, ap=[*ap.ap, [0, n]])


def _emit(ctx: ExitStack, tc: tile.TileContext, h):
    nc = tc.nc

    persist = ctx.enter_context(tc.tile_pool(name="persist", bufs=1))
    consts = ctx.enter_context(tc.tile_pool(name="consts", bufs=1))

    # ---- constants ----
    idt = consts.tile([128, 128], BF16)
    make_identity(nc, idt[:])
    bq_sb = consts.tile([128, NOC], F32)
    nc.gpsimd.dma_start(out=bq_sb[:], in_=h["bq"].ap())
    bk_sb = consts.tile([128, NOC], F32)
    nc.gpsimd.dma_start(out=bk_sb[:], in_=h["bk"].ap())
    bv_sb = consts.tile([1, EC], BF16)
    nc.gpsimd.dma_start(out=bv_sb[:], in_=h["bv"].ap())
    ones1 = consts.tile([1, 128], BF16)
    nc.vector.memset(ones1[:], 1.0)
    scratch1 = consts.tile([1, 1], BF16)
    # dummy exp at t~0: pulls the ACT exp-table load off the critical path
    nc.scalar.activation(scratch1[:], ones1[0:1, 0:1], Exp)

    # ---- persistent activations ----
    qT = persist.tile([128, NOC, S], BF16)        # [o%128, o-chunk, m]
    kT = persist.tile([128, NOC, S], BF16)
    vaug = persist.tile([128, NKC, HPC, D + 4], BF16)  # [m%128, m-chunk, head, d|one]
    ET = persist.tile([128, NKC, S], BF16)        # exp(dmaskT + amask), [k%128, k-chunk, q]

    nc.vector.memset(vaug[:, :, :, D : D + 1], 1.0)

    cstg = ctx.enter_context(tc.tile_pool(name="cstg", bufs=3))

    # stage-D pools open first so their SBUF/PSUM does not overlap the
    # projection pools (avoids release-chain stalls at the phase boundary)
    sps = ctx.enter_context(tc.tile_pool(name="s_psum", bufs=3, space="PSUM"))
    dwork = ctx.enter_context(tc.tile_pool(name="dwork", bufs=4))
    owork = ctx.enter_context(tc.tile_pool(name="owork", bufs=3))

    # ---- stages A+B: load + projections; stage C interleaved ----
    with tc.tile_pool(name="stageAB", bufs=1) as sab, \
         tc.tile_pool(name="proj_psum", bufs=2, space="PSUM") as pps:
        xTb = sab.tile([128, NIC, S], BF16)
        wqb = sab.tile([128, NIC, EC], BF16)
        wkb = sab.tile([128, NIC, EC], BF16)
        wvb = sab.tile([128, NIC, EC], BF16)
        # xT (bf16): straight load on the sync queue
        for c in range(NIC // 2):
            nc.sync.dma_start(
                out=xTb[:, 2 * c : 2 * c + 2, :],
                in_=h["xT"].ap()[c * 256 : (c + 1) * 256, :].rearrange(
                    "(a p) q -> p a q", p=128
                ),
            )
        # W (bf16): gpsimd queue, concurrent with the sync queue
        for name, wtb in (("wqT", wqb), ("wkT", wkb), ("wvT", wvb)):
            for ic in range(NIC):
                nc.gpsimd.dma_start(
                    out=wtb[:, ic, :], in_=h[name].ap()[ic * 128 : (ic + 1) * 128, :]
                )

        def proj_qk(dst, wtb, bias, oc):
            for mq in range(NQQ):
                ps = pps.tile([128, QW], F32, tag="pp")
                for ic in range(NIC):
                    nc.tensor.matmul(
                        ps[:],
                        wtb[:, ic, oc * 128 : (oc + 1) * 128],
                        xTb[:, ic, mq * QW : (mq + 1) * QW],
                        start=(ic == 0),
                        stop=(ic == NIC - 1),
                    )
                nc.vector.tensor_scalar_add(
                    dst[:, oc, mq * QW : (mq + 1) * QW], ps[:], bias[:, oc : oc + 1]
                )

        def proj_v(mc):
            vps_full = pps.tile([128, QW], F32, tag="pp")
            vps = vps_full[:, 0:EC]
            for ic in range(NIC):
                nc.tensor.matmul(
                    vps[:],
                    xTb[:, ic, mc * 128 : (mc + 1) * 128],
                    wvb[:, ic, :],
                    start=(ic == 0),
                    stop=False,
                )
            nc.tensor.matmul(vps[:], ones1[:], bv_sb[:], start=False, stop=True)
            nc.vector.tensor_copy(
                vaug[:, mc, :, 0:D], vps[:].rearrange("p (h d) -> p h d", h=HPC)
            )

        # j=0 needs qT/kT chunk 0 + vaug; emit those first so stage D can start
        proj_qk(qT, wqb, bq_sb, 0)
        proj_qk(kT, wkb, bk_sb, 0)
        for mc in range(NKC):
            proj_v(mc)
        for oc in range(1, NOC):
            proj_qk(qT, wqb, bq_sb, oc)
            proj_qk(kT, wkb, bk_sb, oc)

    # ---- stage C: E_T = exp(maskT) chunks; the first few are emitted up
    # front, the rest interleave into the first k-loop so the score-exp
    # stream isn't queued behind the whole mask-exp chain on ScalarE ----
    def emit_ET(kc):
        dm = cstg.tile([128, S], BF16, tag="dm")
        eng = nc.sync if kc % 2 == 0 else nc.gpsimd
        eng.dma_start(
            out=dm[:], in_=h["dmaskT"].ap()[kc * 128 : (kc + 1) * 128, :]
        )
        nc.scalar.activation(ET[:, kc, :], dm[:], Exp)

    ET_AHEAD = 4
    for kc in range(ET_AHEAD):
        emit_ET(kc)

    # ---- stage D: attention ----
    with tc.tile_pool(name="tailwork", bufs=7) as twork:

        def tail(S_t, kc, j, qs, ctxA, ctxB):
            ex = dwork.tile([128, 2 * QW], BF16, tag="ex")
            nc.scalar.activation(ex[:], S_t[:], Exp)
            pr = dwork.tile([128, 2 * QW], BF16, tag="pr")
            et_ap = ET[:, kc, qs]
            et_b = bass.AP(
                tensor=et_ap.tensor, offset=et_ap.offset,
                ap=[et_ap.ap[0], [0, 2], *et_ap.ap[1:]],
            )
            nc.vector.tensor_tensor(
                pr[:].rearrange("p (g q) -> p g q", g=2),
                ex[:].rearrange("p (g q) -> p g q", g=2),
                et_b,
                op=mybir.AluOpType.mult,
            )
            nc.tensor.matmul(
                ctxA[:], vaug[:, kc, 2 * j, 0 : D + 1], pr[:, 0:QW],
                start=(kc == 0), stop=(kc == NKC - 1),
            )
            nc.tensor.matmul(
                ctxB[:], vaug[:, kc, 2 * j + 1, 0 : D + 1], pr[:, QW : 2 * QW],
                start=(kc == 0), stop=(kc == NKC - 1),
            )

        for qq in range(NQQ):
            qs = slice(qq * QW, (qq + 1) * QW)
            osb_t = owork.tile([128, 4, EC], BF16, tag="osb")
            for j in range(NOC):
                csbs = []
                with tc.tile_pool(name="ctxp", bufs=1, space="PSUM") as cps:
                    ctxA = cps.tile([D + 1, QW], F32, tag="cA")
                    ctxB = cps.tile([D + 1, QW], F32, tag="cB")
                    prev = None
                    for kc in range(NKC):
                        if qq == 0 and j == 0 and kc + ET_AHEAD < NKC:
                            emit_ET(kc + ET_AHEAD)
                        ks = slice(kc * 128, (kc + 1) * 128)
                        S_t = sps.tile([128, 2 * QW], F32, tag="S")
                        nc.tensor.matmul(
                            S_t[:, 0:QW], kT[0:64, j, ks], qT[0:64, j, qs],
                            start=True, stop=True, tile_position=(0, 0),
                        )
                        nc.tensor.matmul(
                            S_t[:, QW : 2 * QW], kT[64:128, j, ks], qT[64:128, j, qs],
                            start=True, stop=True, tile_position=(64, 0),
                        )
                        if prev is not None:
                            tail(prev[0], prev[1], j, qs, ctxA, ctxB)
                        prev = (S_t, kc)
                    tail(prev[0], prev[1], j, qs, ctxA, ctxB)
                    for cpsum in (ctxA, ctxB):
                        csb = twork.tile([D + 1, QW], BF16, tag="csb")
                        nc.vector.tensor_copy(csb[:], cpsum[:])
                        csbs.append(csb)
                with tc.tile_pool(name="tpp", bufs=2, space="PSUM") as tpp:
                    for hh, csb in enumerate(csbs):
                        tp = tpp.tile([128, 4, D + 4], BF16, tag="tp")
                        for t in range(4):
                            nc.tensor.transpose(
                                tp[:, t, 0 : D + 1],
                                csb[:, t * 128 : (t + 1) * 128],
                                idt[0 : D + 1, 0 : D + 1],
                            )
                        rc4 = twork.tile([128, 4], F32, tag="rc4")
                        nc.vector.reciprocal(
                            rc4[:], tp[:, :, D : D + 1].rearrange("p a b -> p (a b)")
                        )
                        col = (2 * j + hh) * D
                        nc.vector.tensor_tensor(
                            osb_t[:, :, col : col + D],
                            tp[:, :, 0:D],
                            _bcast_last(rc4[:], D),
                            op=mybir.AluOpType.mult,
                        )
            nc.gpsimd.dma_start(
                out=h["out"].ap()[qq * QW : (qq + 1) * QW, :].rearrange(
                    "(t p) e -> p t e", p=128
                ),
                in_=osb_t[:],
            )


def build():
    nc = bacc.Bacc("TRN2", target_bir_lowering=False, debug=False, num_devices=N_CORES)
    h = {
        "xT": nc.dram_tensor("xT", [E, S], BF16, kind="ExternalInput"),
        "wqT": nc.dram_tensor("wqT", [E, EC], BF16, kind="ExternalInput"),
        "wkT": nc.dram_tensor("wkT", [E, EC], BF16, kind="ExternalInput"),
        "wvT": nc.dram_tensor("wvT", [E, EC], BF16, kind="ExternalInput"),
        "bq": nc.dram_tensor("bq", [128, NOC], F32, kind="ExternalInput"),
        "bk": nc.dram_tensor("bk", [128, NOC], F32, kind="ExternalInput"),
        "bv": nc.dram_tensor("bv", [1, EC], BF16, kind="ExternalInput"),
        "dmaskT": nc.dram_tensor("dmaskT", [S, S], BF16, kind="ExternalInput"),
        "out": nc.dram_tensor("out", [S, EC], F32, kind="ExternalOutput"),
    }
    with tile.TileContext(nc) as tc:
        with ExitStack() as ctx:
            _emit(ctx, tc, h)
    nc.compile()
    return nc


def prep_in_maps(inputs):
    hs = np.asarray(inputs["hidden_states"], dtype=np.float32)
    am = np.asarray(inputs["attention_mask"], dtype=np.float32)
    dm = np.asarray(inputs["domain_attn_mask"], dtype=np.float32)
    Wq = np.asarray(inputs["Wq"], dtype=np.float32)
    bq = np.asarray(inputs["bq"], dtype=np.float32)
    Wk = np.asarray(inputs["Wk"], dtype=np.float32)
    bk = np.asarray(inputs["bk"], dtype=np.float32)
    Wv = np.asarray(inputs["Wv"], dtype=np.float32)
    bv = np.asarray(inputs["bv"], dtype=np.float32)

    in_maps = []
    for c in range(N_CORES):
        b = c // 2
        e0 = (c % 2) * EC
        sl = slice(e0, e0 + EC)
        in_maps.append(
            {
                "xT": np.ascontiguousarray(hs[b].T).astype(ml_dtypes.bfloat16),
                "wqT": (np.ascontiguousarray(Wq[sl, :].T) * 0.125).astype(
                    ml_dtypes.bfloat16
                ),
                "wkT": np.ascontiguousarray(Wk[sl, :].T).astype(ml_dtypes.bfloat16),
                "wvT": np.ascontiguousarray(Wv[sl, :].T).astype(ml_dtypes.bfloat16),
                "bq": np.ascontiguousarray((bq[sl] * 0.125).reshape(NOC, 128).T),
                "bk": np.ascontiguousarray(bk[sl].reshape(NOC, 128).T),
                "bv": bv[sl].reshape(1, EC).astype(ml_dtypes.bfloat16),
                "dmaskT": (dm[b, 0].T + am[b, 0, 0, :, None]).astype(ml_dtypes.bfloat16),
            }
        )
    return in_maps


_cached_nc = None


def run(inputs, trace=False):
    global _cached_nc
    if _cached_nc is None:
        _cached_nc = build()
    in_maps = prep_in_maps(inputs)
    res = run_bass_kernel_spmd(
        _cached_nc, in_maps, core_ids=list(range(N_CORES)), trace=trace
    )
    out = np.empty((B, S, E), dtype=np.float32)
    for c in range(N_CORES):
        b = c // 2
        e0 = (c % 2) * EC
        out[b, :, e0 : e0 + EC] = res.results[c]["out"]
    return out, res


def kernel(**inputs) -> np.ndarray:
    return run(inputs)[0]
